# revision 52
# baseline (speedup 1.0000x reference)
"""Trainium2 Bass kernel: CLUTRR-style GNN message passing (nn_CLUTRRV4).

Data-parallel across 8 NeuronCores. Samples are packed 4-per-group
(4 x 32 entity slots = 128 partitions); sample->group assignment is an
LPT bin-packing so that each group's VALID edges fit in EC=128 packed
edge columns (vs 256 naive), skipping all masked-edge compute.

All one-hot gather/scatter/rel matrices are precomputed on the host and
DMA'd once (they are step-invariant); nothing is generated on-chip.
Entity state S is fp16-only (tolerance 2e-2 >> fp16 error here).

Per step, per group: S is transposed (PE) to slot-major, src/tgt states
are gathered via one-hot matmuls, the message MLP layer 1 uses fixed
weight blocks (rel contribution via the 20-row band trick), messages are
scattered back with the edge-major one-hot, and the update MLP runs per
quad (4 groups) with N=512 matmuls. Emission is a software-pipelined
flat loop (modulo schedule) so the PE never waits on the DVE/Act
converts; PSUM is budgeted at exactly 8 banks.
"""
import sys
import numpy as np

if "/opt/trn_rl_repo" not in sys.path:
    sys.path.append("/opt/trn_rl_repo")

N_ENT, N_REL, D, E = 32, 20, 128, 64
N_STEPS = 8
N_CORES = 8
P = 128
EC = 128          # packed edge columns per group
GRP = 4           # samples per group
STT_AGB = True
REL_BANDS = False



def _build_nc(G, n_steps):
    from concourse import bacc, mybir
    from concourse.tile import TileContext
    from concourse.masks import make_identity

    f32 = mybir.dt.float32
    f16 = mybir.dt.float16
    AF = mybir.ActivationFunctionType
    OP = mybir.AluOpType

    assert G % 4 == 0
    NQ = G // 4
    SLOTS = G * P

    nc = bacc.Bacc()

    def din(name, shape, dtype=f32):
        return nc.declare_dram_parameter(name, list(shape), dtype, isOutput=False)

    NP = G // 2
    d_s0 = din("s0", (P, SLOTS), f16)
    d_oh = din("oh", (NQ, P, 12 * EC), f16)     # 4 groups x [ohs|oht|ohe]
    d_relt4 = din("relt4", (P, NQ * 512), f16)  # quad-wide rel one-hots
    d_cntb = din("cntb", (P, G * P), f16)       # per-slot edge counts, bcast
    d_qoh = din("qoh", (P, G * 8), f16)
    d_sts0 = din("sts0", (P, P), f16)           # slot-major initial state
    d_wf16 = din("wf16", (P, 2176), f16)
    d_wf32 = din("wf32", (P, 281))
    d_cb2 = din("cb2", (20, 1))
    d_out = nc.declare_dram_parameter("out", [20, G * GRP], f32, isOutput=True)

    with TileContext(nc) as tc:
        with (
            tc.tile_pool(name="c", bufs=1) as cp,
            tc.tile_pool(name="w", bufs=4) as wp,
            tc.tile_pool(name="pGA", bufs=1, space="PSUM") as pGA,
            tc.tile_pool(name="pH1", bufs=1, space="PSUM") as pH1,
            tc.tile_pool(name="pMS", bufs=1, space="PSUM") as pMS,
            tc.tile_pool(name="pAG", bufs=1, space="PSUM") as pAG,
            tc.tile_pool(name="pUP", bufs=2, space="PSUM") as pUP,
            tc.tile_pool(name="pTR", bufs=1, space="PSUM") as pTR,
            tc.tile_pool(name="pH2", bufs=1, space="PSUM") as pH2,
        ):
            wf16 = cp.tile([P, 2176], f16, tag="wf16", name="wf16")
            w1s = wf16[:, 0:256]
            w1t = wf16[:, 256:512]
            rt4 = wf16[:, 512:768]
            w2m = wf16[:, 768:1024]
            w1u = wf16[:, 1024:1536]
            w2u = wf16[:, 1536:1792]
            b2row = wf16[:, 1792:1920]
            cwst = wf16[:, 1920:2176]
            wf32 = cp.tile([P, 281], f32, tag="wf32", name="wf32")
            nc.sync.dma_start(wf32[:], d_wf32[:])
            b1u = wf32[:, 0:2]
            b2u = wf32[:, 2:3]
            cw1 = wf32[:, 3:259]
            cb1 = wf32[:, 259:260]
            cw2 = wf32[:, 260:280]
            b2mc = wf32[:, 280:281]
            cb2 = cp.tile([20, 1], f32, tag="cb2", name="cb2")
            nc.sync.dma_start(cb2[:], d_cb2[:])

            ident = cp.tile([P, P], f16, tag="ident", name="ident")
            make_identity(nc, ident[:])

            # interleave S-quad and one-hot-quad DMAs so step-0 compute
            # never starves; rel/ind early (needed at h1/sc offsets)
            S = cp.tile([P, SLOTS], f16, tag="S", name="S")
            OH = cp.tile([P, G * 3 * EC], f16, tag="OH", name="OH")
            RELT4 = cp.tile([P, NQ * 512], f16, tag="RELT4", name="RELT4")
            CNTB = cp.tile([P, G * P], f16, tag="CNTB", name="CNTB")
            # step-0-critical tiles first: sts0 (shared t=0 slot-major
            # state) + OH/RELT2 chunks; S (first read by up, ~10 units in)
            # and CNTB (first read by sc) trail each quad's critical pair
            sts0 = cp.tile([P, P], f16, tag="sts0", name="sts0")
            nc.sync.dma_start(sts0[:], d_sts0[:])
            nc.sync.dma_start(OH[:, 0:1536], d_oh[0])
            nc.sync.dma_start(wf16[:], d_wf16[:])
            for q in range(NQ):
                if q > 0:
                    nc.sync.dma_start(OH[:, q * 1536:(q + 1) * 1536],
                                      d_oh[q])
                nc.sync.dma_start(RELT4[:, q * 512:(q + 1) * 512],
                                  d_relt4[:, q * 512:(q + 1) * 512])
                nc.sync.dma_start(S[:, q * 512:(q + 1) * 512],
                                  d_s0[:, q * 512:(q + 1) * 512])
                nc.sync.dma_start(CNTB[:, q * 512:(q + 1) * 512],
                                  d_cntb[:, q * 512:(q + 1) * 512])
            qoh = cp.tile([P, G * 8], f16, tag="qoh", name="qoh")
            nc.sync.dma_start(qoh[:], d_qoh[:])

            outsb = cp.tile([20, G * GRP], f32, tag="outsb", name="outsb")

            def ohs(g):
                return OH[:, g * 384:g * 384 + EC]

            def oht(g):
                return OH[:, g * 384 + EC:g * 384 + 2 * EC]

            def ohe(g):
                return OH[:, g * 384 + 2 * EC:g * 384 + 3 * EC]

            mm = nc.tensor.matmul
            st_ = {}
            sts_t, gtb_t, h1g_t, msb_t, agb_t = {}, {}, {}, {}, {}

            # --- pipeline stages ------------------------------------------
            def st_stage(t, g):
                # slot-major S replica via sync-queue DMA XBAR transpose.
                # At t=0 every group's S is the same tiled entity table ->
                # share one host-sent sts0 tile.
                if t == 0:
                    sts_t[g] = sts0
                    return
                t_ = wp.tile([P, P], f16, tag="sts", bufs=8, name="sts")
                if g % 2 == 0:
                    nc.sync.dma_start_transpose(t_[:], S[:, g * P:(g + 1) * P])
                else:
                    tp = pTR.tile([P, P], f16, tag="tr", name="tp")
                    mm(tp[:], lhsT=S[:, g * P:(g + 1) * P], rhs=ident[:],
                       is_transpose=True, start=True, stop=True)
                    nc.scalar.copy(t_[:], tp[:])
                sts_t[g] = t_

            def ga_stage(t, g):
                gi = g % 2
                if g % 4 == 0:
                    gtb_t[g // 4] = wp.tile([P, 1024], f16, tag="gtb",
                                            name="gtb")
                if gi == 0:
                    st_['gap'] = pGA.tile([P, 512], f32, tag="ga", name="gap")
                gp = st_['gap']
                mm(gp[:, gi * 256:(gi + 1) * 256],
                   lhsT=sts_t[g][:], rhs=OH[:, g * 384:g * 384 + 2 * EC],
                   start=True, stop=True)
                if gi == 1:
                    p = g // 2
                    gtb = gtb_t[g // 4]
                    nc.vector.tensor_copy(gtb[:, (p % 2) * 512:
                                              (p % 2) * 512 + 512], gp[:])

            def h1_stage(t, g):
                # quad-wide: 6 mms of N=512 per 4 groups. gtb layout is
                # [g0s g0t g1s g1t g2s g2t g3s g3t]; strided AP views pick
                # the four src (resp tgt) blocks as the moving operand.
                if g % 4 != 3:
                    return
                q = g // 4
                rb = (q % 2) * 32
                gtb8 = gtb_t[q][:].rearrange("p (g s e) -> p g s e",
                                             g=4, s=2, e=EC)
                for c in (0, 1):
                    pool, tg = ((pH1, "h1") if c == 0 else (pH2, "h1b"))
                    hp = pool.tile([P, 512], f32, tag=tg, name="h1p")
                    mm(hp[:], lhsT=rt4[rb:rb + 20, c * P:(c + 1) * P],
                       rhs=RELT4[rb:rb + 20, q * 512:(q + 1) * 512],
                       start=True, stop=False, tile_position=(rb, 0))
                    mm(hp[:], lhsT=w1s[:, c * P:(c + 1) * P],
                       rhs=gtb8[:, :, 0, :], start=False, stop=False)
                    mm(hp[:], lhsT=w1t[:, c * P:(c + 1) * P],
                       rhs=gtb8[:, :, 1, :], start=False, stop=True)
                    t_ = wp.tile([P, 512], f16, tag="h1g", name="h1g")
                    nc.scalar.activation(t_[:], hp[:], AF.Gelu)
                    h1g_t[q * 2 + c] = t_

            def ms_stage(t, g):
                q, j = divmod(g, 4)
                if j == 0:
                    st_['msp'] = pMS.tile([P, 512], f32, tag="ms", name="msp")
                mp = st_['msp']
                o = mp[:, j * P:(j + 1) * P]
                for c in (0, 1):
                    h1g = h1g_t[q * 2 + c]
                    mm(o, lhsT=h1g[:, j * P:(j + 1) * P],
                       rhs=w2m[:, c * P:(c + 1) * P],
                       start=(c == 0), stop=(c == 1))
                if j == 3:
                    t_ = wp.tile([P, 512], f16, tag="msb", name="msb")
                    nc.vector.tensor_copy(t_[:], mp[:])
                    msb_t[g // 4] = t_

            def sc_stage(t, g):
                q, j = divmod(g, 4)
                if j == 0:
                    st_['agp'] = pAG.tile([P, 512], f32, tag="ag", name="agp")
                ap_ = st_['agp']
                msb = msb_t[q]
                o = ap_[:, j * P:(j + 1) * P]
                mm(o, lhsT=msb[:, j * P:(j + 1) * P], rhs=ohe(g),
                   start=True, stop=True)
                if j == 3:
                    # agb = agg + msg_b2 * per-slot valid-edge count, folded
                    # into the PSUM->SBUF copy (no PE b2 matmuls)
                    t_ = wp.tile([P, 512], f16, tag="agb", name="agb")
                    if STT_AGB:
                        nc.vector.scalar_tensor_tensor(
                            out=t_[:], in0=ap_[:], scalar=1.0,
                            in1=CNTB[:, q * 512:(q + 1) * 512],
                            op0=OP.mult, op1=OP.add)
                    else:
                        nc.scalar.copy(t_[:], ap_[:])
                    agb_t[q] = t_

            def up_stage(t, w):
                q, ph = divmod(w, 4)
                if ph == 0:
                    h3a = pUP.tile([P, 512], f32, tag="up", name="h3a")
                    h3b = pUP.tile([P, 512], f32, tag="up", name="h3b")
                    st_['h3a'], st_['h3b'] = h3a, h3b
                    for mc, hb in ((0, h3a), (1, h3b)):
                        mm(hb[:], lhsT=w1u[:, mc * P:(mc + 1) * P],
                           rhs=S[:, q * 512:(q + 1) * 512],
                           start=True, stop=False)
                        mm(hb[:], lhsT=w1u[:, 256 + mc * P:256 + (mc + 1) * P],
                           rhs=agb_t[q][:], start=False, stop=True)
                elif ph == 1:
                    t_ = wp.tile([P, 1024], f16, tag="h3g", name="h3g")
                    nc.scalar.activation(t_[:, 0:512], st_['h3a'][:], AF.Gelu,
                                         bias=b1u[:, 0:1])
                    st_['h3g'] = t_
                elif ph == 2:
                    nc.scalar.activation(st_['h3g'][:, 512:1024],
                                         st_['h3b'][:], AF.Gelu,
                                         bias=b1u[:, 1:2])
                else:
                    sn = pUP.tile([P, 512], f32, tag="up", name="sn")
                    for kc in (0, 1):
                        mm(sn[:], lhsT=w2u[:, kc * P:(kc + 1) * P],
                           rhs=st_['h3g'][:, kc * 512:(kc + 1) * 512],
                           start=(kc == 0), stop=(kc == 1))
                    nc.vector.scalar_tensor_tensor(
                        out=S[:, q * 512:(q + 1) * 512], in0=sn[:],
                        scalar=b2u[:, 0:1], in1=S[:, q * 512:(q + 1) * 512],
                        op0=OP.add, op1=OP.add)

            # --- software-pipelined flat loop -----------------------------
            # Stages are emitted deepest-offset first within each unit so
            # that every read of a rotating tile precedes the alloc of the
            # generation that reuses its buffer (WAR legality).
            # In-unit order: stall-prone update mms go LAST (in-order PE:
            # a waiting sn mm would block every later mm in the unit), and
            # h1g gelus land early in the Act queue. WAR legality needs
            # ms < h1 < ga.
            assert G >= 32, "modulo-schedule offsets need G >= 32"
            offs = (27, 21, 16, 10, 6, 0)
            stages = (up_stage, sc_stage, ms_stage, h1_stage, ga_stage,
                      st_stage)
            total = n_steps * G + offs[0] + 4
            for U in range(total):
                for off, fn in zip(offs, stages):
                    v = U - off
                    if v < 0:
                        continue
                    t, g = divmod(v, G)
                    if t < n_steps:
                        fn(t, g)

            # --- classifier head (transpose-free A-form) ------------------
            # Ac = S_g^T @ [cw1_s | cw1_t]  ->  [slots, 256];  then per-group
            # one-hot row-selects build hq[hid, 4] = cw1_s^T S[qs] +
            # cw1_t^T S[qt]; cb1 folds into the gelu bias (per-partition).
            nbatch = (G + 15) // 16
            for bq in range(nbatch):
                jn = min(16, G - bq * 16)
                pq = pMS.tile([P, 64], f32, tag="ms", name="pq")
                for j in range(jn):
                    g = bq * 16 + j
                    acp, act = ((pH1, "h1"), (pH2, "h1b"),
                                (pGA, "ga"))[j % 3]
                    ac = acp.tile([P, 256], f32, tag=act, name="ac")
                    mm(ac[:], lhsT=S[:, g * P:(g + 1) * P], rhs=cwst[:],
                       start=True, stop=True)
                    acs = wp.tile([P, 256], f16, tag="acs", name="acs")
                    if j % 2 == 0:
                        nc.vector.tensor_copy(acs[:], ac[:])
                    else:
                        nc.scalar.copy(acs[:], ac[:])
                    mm(pq[:, j * 4:(j + 1) * 4], lhsT=acs[:, 0:128],
                       rhs=qoh[:, g * 8:g * 8 + 4], start=True, stop=False)
                    mm(pq[:, j * 4:(j + 1) * 4], lhsT=acs[:, 128:256],
                       rhs=qoh[:, g * 8 + 4:g * 8 + 8],
                       start=False, stop=True)
                hg = wp.tile([P, 64], f32, tag="hg", name="hg")
                nc.scalar.activation(hg[:, 0:jn * 4], pq[:, 0:jn * 4],
                                     AF.Gelu, bias=cb1[:, 0:1])
                ops_ = pAG.tile([20, 64], f32, tag="ag", name="ops")
                mm(ops_[:, 0:jn * 4], lhsT=cw2[:], rhs=hg[:, 0:jn * 4],
                   start=True, stop=True)
                nc.scalar.activation(outsb[:, bq * 64:bq * 64 + jn * 4],
                                     ops_[:, 0:jn * 4], AF.Identity,
                                     bias=cb2[:, 0:1])
            nc.sync.dma_start(d_out[:], outsb[:])

    nc.finalize()
    return nc


def _assign_groups(ne, n_groups):
    """LPT bin packing: samples -> groups of <=GRP samples, balancing edge
    counts. Returns (group_of_sample, slot_of_sample, max_load)."""
    import heapq
    B = ne.shape[0]
    order = np.argsort(-ne, kind="stable")
    loads = [0] * n_groups
    counts = [0] * n_groups
    gof = np.zeros(B, np.int64)
    sof = np.zeros(B, np.int64)
    hp = [(0, g) for g in range(n_groups)]
    heapq.heapify(hp)
    for s in order:
        while True:
            ld, g = heapq.heappop(hp)
            if counts[g] < GRP and ld == loads[g]:
                break
        gof[s] = g
        sof[s] = counts[g]
        counts[g] += 1
        loads[g] += int(ne[s])
        if counts[g] < GRP:
            heapq.heappush(hp, (loads[g], g))
    return gof, sof, max(loads)


def _host_prep(inputs, G=None):
    f, hh = np.float32, np.float16
    src = np.asarray(inputs["edge_src"], np.int64)
    rel = np.asarray(inputs["edge_rel"], np.int64)
    tgt = np.asarray(inputs["edge_tgt"], np.int64)
    ne = np.asarray(inputs["n_edges"], np.int64)
    qs = np.asarray(inputs["query_src"], np.int64)
    qt = np.asarray(inputs["query_tgt"], np.int64)
    B = src.shape[0]
    if G is None:
        bc = -(-B // N_CORES)
        G = max(32, (-(-bc // GRP) + 3) // 4 * 4)
    while True:
        gof, sof, maxload = _assign_groups(ne, G * N_CORES)
        if maxload <= EC:
            break
        G += 4
    NG = G * N_CORES
    NQ = G // 4
    NP = G // 2

    oh = np.zeros((NG, P, 3 * EC), hh)
    relt4 = np.zeros((NG // 4, P, 512), hh)
    cnt = np.zeros(NG * P, np.float32)
    qoh = np.zeros((NG, P, 8), hh)
    ecnt = np.zeros(NG, np.int64)
    for s in range(B):
        g = int(gof[s])
        so = int(sof[s]) * N_ENT
        k = int(ne[s])
        qg, m4 = divmod(g, 4)
        rb = ((qg % (G // 4)) % 2) * 32
        if k:
            e0 = int(ecnt[g])
            ecnt[g] += k
            idx = np.arange(e0, e0 + k)
            es, et, er = src[s, :k], tgt[s, :k], rel[s, :k]
            oh[g, so + es, idx] = 1
            oh[g, so + et, EC + idx] = 1
            oh[g, idx, 2 * EC + so + et] = 1
            relt4[qg, rb + er, m4 * EC + idx] = 1
            np.add.at(cnt, g * P + so + et, 1.0)
        qoh[g, so + qs[s], sof[s]] = 1
        qoh[g, so + qt[s], 4 + sof[s]] = 1
    cntb = (np.asarray(inputs["msg_b2"], f).reshape(P, 1)
            * cnt[None, :]).astype(hh)

    # shared weights
    ee = np.asarray(inputs["entity_embed"], f)
    W1 = np.asarray(inputs["msg_W1"], f)
    reltab = (np.asarray(inputs["rel_embed"], f) @ W1[128:256]
              + np.asarray(inputs["msg_b1"], f))
    rt4 = np.zeros((P, 256), f)
    b2row = np.zeros((P, P), f)
    for rb in (0, 32, 64, 96):
        rt4[rb:rb + 20] = reltab
        b2row[rb] = np.asarray(inputs["msg_b2"], f)
    w2m_ = np.asarray(inputs["msg_W2"], f)
    w1u_ = np.asarray(inputs["upd_W1"], f)
    w2u_ = np.asarray(inputs["upd_W2"], f)
    cw1_ = np.asarray(inputs["cls_W1"], f)

    wf16 = np.concatenate([
        W1[0:128], W1[256:384], rt4,
        np.concatenate([w2m_[0:128], w2m_[128:256]], axis=1),
        np.concatenate(
            [w1u_[0:128, 0:128], w1u_[0:128, 128:256],
             w1u_[128:256, 0:128], w1u_[128:256, 128:256]], axis=1),
        np.concatenate([w2u_[0:128], w2u_[128:256]], axis=1),
        b2row,
        np.concatenate([cw1_[0:128], cw1_[128:256]], axis=1),
    ], axis=1).astype(hh)
    wf32 = np.concatenate([
        np.asarray(inputs["upd_b1"], f).reshape(2, 128).T,
        np.asarray(inputs["upd_b2"], f).reshape(128, 1),
        np.concatenate([cw1_[0:128], cw1_[128:256]], axis=1),
        np.asarray(inputs["cls_b1"], f).reshape(128, 1),
        np.asarray(inputs["cls_W2"], f),
        np.asarray(inputs["msg_b2"], f).reshape(128, 1),
    ], axis=1).astype(f)
    shared = {
        "s0": np.tile(ee.T, (1, GRP * G)).astype(hh),
        "sts0": np.tile(ee, (GRP, 1)).astype(hh),
        "wf16": wf16,
        "wf32": wf32,
        "cb2": np.asarray(inputs["cls_b2"], f).reshape(20, 1).copy(),
    }

    in_maps = []
    for c in range(N_CORES):
        gsl = slice(c * G, (c + 1) * G)
        psl = slice(c * NP, (c + 1) * NP)
        m = dict(shared)
        m["oh"] = np.ascontiguousarray(
            oh[gsl].reshape(NQ, 4, P, 3 * EC).transpose(0, 2, 1, 3)
            .reshape(NQ, P, 12 * EC))
        m["relt4"] = np.ascontiguousarray(
            relt4[c * NQ:(c + 1) * NQ].transpose(1, 0, 2)
            .reshape(P, NQ * 512))
        m["cntb"] = np.ascontiguousarray(
            cntb[:, c * G * P:(c + 1) * G * P])
        m["qoh"] = np.ascontiguousarray(
            qoh[gsl].transpose(1, 0, 2).reshape(P, G * 8))
        in_maps.append(m)
    return in_maps, gof, sof, G


_CACHE = {}


def kernel(**inputs):
    B = np.asarray(inputs["edge_src"]).shape[0]
    in_maps, gof, sof, G = _host_prep(inputs)

    key = G
    if key not in _CACHE:
        _CACHE[key] = _build_nc(G, N_STEPS)
    nc = _CACHE[key]

    from concourse.bass_utils import run_bass_kernel_spmd
    res = run_bass_kernel_spmd(nc, in_maps, core_ids=list(range(N_CORES)))

    out = np.empty((B, N_REL), np.float32)
    allc = np.concatenate([r["out"].T for r in res.results], axis=0)
    # row index in allc: core * (G*4) + (g_local*4 + slot) = gof*4 + sof
    out[:, :] = allc[gof * 4 + sof]
    return np.ascontiguousarray(out)



# revision 53
# speedup vs baseline: 1.0106x; 1.0106x over previous
"""Trainium2 Bass kernel: CLUTRR-style GNN message passing (nn_CLUTRRV4).

Data-parallel across 8 NeuronCores. Samples are packed 4-per-group
(4 x 32 entity slots = 128 partitions); sample->group assignment is an
LPT bin-packing so that each group's VALID edges fit in EC=128 packed
edge columns (vs 256 naive), skipping all masked-edge compute.

All one-hot gather/scatter/rel matrices are precomputed on the host and
DMA'd once (they are step-invariant). Entity state S is fp16.

Per step: S is transposed to slot-major (sync-queue DMA XBAR transpose
alternating with PE is_transpose matmuls; step 0 shares one host-sent
tile since S starts identical for every group), src/tgt states are
gathered via one-hot matmuls into a quad-wide gtb, the message MLP
layer 1 runs quad-wide (6 mms of N=512 per 4 groups, rel contribution
via a 20-row band), messages are scattered back edge-major, the msg_b2
bias folds into the PSUM->SBUF copy as a DVE scalar_tensor_tensor with
a host-precomputed b2*edge-count table, and the update MLP runs per
quad with N=512 matmuls. The classifier head is transpose-free
(A-form: Ac = S_g^T [cw1_s|cw1_t], then one-hot row-selects; cb1 rides
the gelu bias). Emission is a software-pipelined flat loop (modulo
schedule); PSUM is budgeted at exactly 8 banks.
"""
import sys
import numpy as np

if "/opt/trn_rl_repo" not in sys.path:
    sys.path.append("/opt/trn_rl_repo")

N_ENT, N_REL, D, E = 32, 20, 128, 64
N_STEPS = 8
N_CORES = 8
P = 128
EC = 128          # packed edge columns per group
GRP = 4           # samples per group



def _build_nc(G, n_steps):
    from concourse import bacc, mybir
    from concourse.tile import TileContext
    from concourse.masks import make_identity

    f32 = mybir.dt.float32
    f16 = mybir.dt.float16
    AF = mybir.ActivationFunctionType
    OP = mybir.AluOpType

    assert G % 4 == 0
    NQ = G // 4
    SLOTS = G * P

    nc = bacc.Bacc()

    def din(name, shape, dtype=f32):
        return nc.declare_dram_parameter(name, list(shape), dtype, isOutput=False)

    NP = G // 2
    d_s0 = din("s0", (P, SLOTS), f16)
    d_oh = din("oh", (NQ, P, 12 * EC), f16)     # 4 groups x [ohs|oht|ohe]
    d_relt4 = din("relt4", (P, NQ * 512), f16)  # quad-wide rel one-hots
    d_cntb = din("cntb", (P, G * P), f16)       # per-slot edge counts, bcast
    d_qoh = din("qoh", (P, G * 8), f16)
    d_sts0 = din("sts0", (P, P), f16)           # slot-major initial state
    d_wf16 = din("wf16", (P, 2176), f16)
    d_wf32 = din("wf32", (P, 281))
    d_cb2 = din("cb2", (20, 1))
    d_out = nc.declare_dram_parameter("out", [20, G * GRP], f32, isOutput=True)

    with TileContext(nc) as tc:
        with (
            tc.tile_pool(name="c", bufs=1) as cp,
            tc.tile_pool(name="w", bufs=4) as wp,
            tc.tile_pool(name="pGA", bufs=1, space="PSUM") as pGA,
            tc.tile_pool(name="pH1", bufs=1, space="PSUM") as pH1,
            tc.tile_pool(name="pMS", bufs=1, space="PSUM") as pMS,
            tc.tile_pool(name="pAG", bufs=1, space="PSUM") as pAG,
            tc.tile_pool(name="pUP", bufs=2, space="PSUM") as pUP,
            tc.tile_pool(name="pTR", bufs=1, space="PSUM") as pTR,
            tc.tile_pool(name="pH2", bufs=1, space="PSUM") as pH2,
        ):
            wf16 = cp.tile([P, 2176], f16, tag="wf16", name="wf16")
            w1s = wf16[:, 0:256]
            w1t = wf16[:, 256:512]
            rt4 = wf16[:, 512:768]
            w2m = wf16[:, 768:1024]
            w1u = wf16[:, 1024:1536]
            w2u = wf16[:, 1536:1792]
            b2row = wf16[:, 1792:1920]
            cwst = wf16[:, 1920:2176]
            wf32 = cp.tile([P, 281], f32, tag="wf32", name="wf32")
            nc.sync.dma_start(wf32[:], d_wf32[:])
            b1u = wf32[:, 0:2]
            b2u = wf32[:, 2:3]
            cw1 = wf32[:, 3:259]
            cb1 = wf32[:, 259:260]
            cw2 = wf32[:, 260:280]
            b2mc = wf32[:, 280:281]
            cb2 = cp.tile([20, 1], f32, tag="cb2", name="cb2")
            nc.sync.dma_start(cb2[:], d_cb2[:])

            ident = cp.tile([P, P], f16, tag="ident", name="ident")
            make_identity(nc, ident[:])

            # interleave S-quad and one-hot-quad DMAs so step-0 compute
            # never starves; rel/ind early (needed at h1/sc offsets)
            S = cp.tile([P, SLOTS], f16, tag="S", name="S")
            OH = cp.tile([P, G * 3 * EC], f16, tag="OH", name="OH")
            RELT4 = cp.tile([P, NQ * 512], f16, tag="RELT4", name="RELT4")
            CNTB = cp.tile([P, G * P], f16, tag="CNTB", name="CNTB")
            # step-0-critical tiles first: sts0 (shared t=0 slot-major
            # state) + OH/RELT2 chunks; S (first read by up, ~10 units in)
            # and CNTB (first read by sc) trail each quad's critical pair
            sts0 = cp.tile([P, P], f16, tag="sts0", name="sts0")
            nc.sync.dma_start(sts0[:], d_sts0[:])
            nc.sync.dma_start(OH[:, 0:1536], d_oh[0])
            nc.sync.dma_start(wf16[:], d_wf16[:])
            for q in range(NQ):
                if q > 0:
                    nc.sync.dma_start(OH[:, q * 1536:(q + 1) * 1536],
                                      d_oh[q])
                nc.sync.dma_start(RELT4[:, q * 512:(q + 1) * 512],
                                  d_relt4[:, q * 512:(q + 1) * 512])
                nc.sync.dma_start(S[:, q * 512:(q + 1) * 512],
                                  d_s0[:, q * 512:(q + 1) * 512])
                nc.sync.dma_start(CNTB[:, q * 512:(q + 1) * 512],
                                  d_cntb[:, q * 512:(q + 1) * 512])
            qoh = cp.tile([P, G * 8], f16, tag="qoh", name="qoh")
            nc.sync.dma_start(qoh[:], d_qoh[:])

            outsb = cp.tile([20, G * GRP], f32, tag="outsb", name="outsb")

            def ohs(g):
                return OH[:, g * 384:g * 384 + EC]

            def oht(g):
                return OH[:, g * 384 + EC:g * 384 + 2 * EC]

            def ohe(g):
                return OH[:, g * 384 + 2 * EC:g * 384 + 3 * EC]

            mm = nc.tensor.matmul
            st_ = {}
            sts_t, gtb_t, h1g_t, msb_t, agb_t = {}, {}, {}, {}, {}

            # --- pipeline stages ------------------------------------------
            def st_stage(t, g):
                # slot-major S replica via sync-queue DMA XBAR transpose.
                # At t=0 every group's S is the same tiled entity table ->
                # share one host-sent sts0 tile.
                if t == 0:
                    sts_t[g] = sts0
                    return
                t_ = wp.tile([P, P], f16, tag="sts", bufs=8, name="sts")
                if g % 2 == 0:
                    nc.sync.dma_start_transpose(t_[:], S[:, g * P:(g + 1) * P])
                else:
                    tp = pTR.tile([P, P], f16, tag="tr", name="tp")
                    mm(tp[:], lhsT=S[:, g * P:(g + 1) * P], rhs=ident[:],
                       is_transpose=True, start=True, stop=True)
                    nc.scalar.copy(t_[:], tp[:])
                sts_t[g] = t_

            def ga_stage(t, g):
                gi = g % 2
                if g % 4 == 0:
                    gtb_t[g // 4] = wp.tile([P, 1024], f16, tag="gtb",
                                            name="gtb")
                if gi == 0:
                    st_['gap'] = pGA.tile([P, 512], f32, tag="ga", name="gap")
                gp = st_['gap']
                mm(gp[:, gi * 256:(gi + 1) * 256],
                   lhsT=sts_t[g][:], rhs=OH[:, g * 384:g * 384 + 2 * EC],
                   start=True, stop=True)
                if gi == 1:
                    p = g // 2
                    gtb = gtb_t[g // 4]
                    nc.vector.tensor_copy(gtb[:, (p % 2) * 512:
                                              (p % 2) * 512 + 512], gp[:])

            def h1_stage(t, g):
                # quad-wide: 6 mms of N=512 per 4 groups. gtb layout is
                # [g0s g0t g1s g1t g2s g2t g3s g3t]; strided AP views pick
                # the four src (resp tgt) blocks as the moving operand.
                if g % 4 != 3:
                    return
                q = g // 4
                rb = (q % 2) * 32
                gtb8 = gtb_t[q][:].rearrange("p (g s e) -> p g s e",
                                             g=4, s=2, e=EC)
                for c in (0, 1):
                    pool, tg = ((pH1, "h1") if c == 0 else (pH2, "h1b"))
                    hp = pool.tile([P, 512], f32, tag=tg, name="h1p")
                    mm(hp[:], lhsT=rt4[rb:rb + 20, c * P:(c + 1) * P],
                       rhs=RELT4[rb:rb + 20, q * 512:(q + 1) * 512],
                       start=True, stop=False, tile_position=(rb, 0))
                    mm(hp[:], lhsT=w1s[:, c * P:(c + 1) * P],
                       rhs=gtb8[:, :, 0, :], start=False, stop=False)
                    mm(hp[:], lhsT=w1t[:, c * P:(c + 1) * P],
                       rhs=gtb8[:, :, 1, :], start=False, stop=True)
                    t_ = wp.tile([P, 512], f16, tag="h1g", name="h1g")
                    nc.scalar.activation(t_[:], hp[:], AF.Gelu)
                    h1g_t[q * 2 + c] = t_

            def ms_stage(t, g):
                q, j = divmod(g, 4)
                if j == 0:
                    st_['msp'] = pMS.tile([P, 512], f32, tag="ms", name="msp")
                mp = st_['msp']
                o = mp[:, j * P:(j + 1) * P]
                for c in (0, 1):
                    h1g = h1g_t[q * 2 + c]
                    mm(o, lhsT=h1g[:, j * P:(j + 1) * P],
                       rhs=w2m[:, c * P:(c + 1) * P],
                       start=(c == 0), stop=(c == 1))
                if j == 3:
                    t_ = wp.tile([P, 512], f16, tag="msb", name="msb")
                    nc.vector.tensor_copy(t_[:], mp[:])
                    msb_t[g // 4] = t_

            def sc_stage(t, g):
                q, j = divmod(g, 4)
                if j == 0:
                    st_['agp'] = pAG.tile([P, 512], f32, tag="ag", name="agp")
                ap_ = st_['agp']
                msb = msb_t[q]
                o = ap_[:, j * P:(j + 1) * P]
                mm(o, lhsT=msb[:, j * P:(j + 1) * P], rhs=ohe(g),
                   start=True, stop=True)
                if j == 3:
                    # agb = agg + msg_b2 * per-slot valid-edge count, folded
                    # into the PSUM->SBUF copy (no PE b2 matmuls)
                    t_ = wp.tile([P, 512], f16, tag="agb", name="agb")
                    nc.vector.scalar_tensor_tensor(
                        out=t_[:], in0=ap_[:], scalar=1.0,
                        in1=CNTB[:, q * 512:(q + 1) * 512],
                        op0=OP.mult, op1=OP.add)
                    agb_t[q] = t_

            def up_stage(t, w):
                q, ph = divmod(w, 4)
                if ph == 0:
                    h3a = pUP.tile([P, 512], f32, tag="up", name="h3a")
                    h3b = pUP.tile([P, 512], f32, tag="up", name="h3b")
                    st_['h3a'], st_['h3b'] = h3a, h3b
                    for mc, hb in ((0, h3a), (1, h3b)):
                        mm(hb[:], lhsT=w1u[:, mc * P:(mc + 1) * P],
                           rhs=S[:, q * 512:(q + 1) * 512],
                           start=True, stop=False)
                        mm(hb[:], lhsT=w1u[:, 256 + mc * P:256 + (mc + 1) * P],
                           rhs=agb_t[q][:], start=False, stop=True)
                elif ph == 1:
                    t_ = wp.tile([P, 1024], f16, tag="h3g", name="h3g")
                    nc.scalar.activation(t_[:, 0:512], st_['h3a'][:], AF.Gelu,
                                         bias=b1u[:, 0:1])
                    st_['h3g'] = t_
                elif ph == 2:
                    nc.scalar.activation(st_['h3g'][:, 512:1024],
                                         st_['h3b'][:], AF.Gelu,
                                         bias=b1u[:, 1:2])
                else:
                    sn = pUP.tile([P, 512], f32, tag="up", name="sn")
                    for kc in (0, 1):
                        mm(sn[:], lhsT=w2u[:, kc * P:(kc + 1) * P],
                           rhs=st_['h3g'][:, kc * 512:(kc + 1) * 512],
                           start=(kc == 0), stop=(kc == 1))
                    nc.vector.scalar_tensor_tensor(
                        out=S[:, q * 512:(q + 1) * 512], in0=sn[:],
                        scalar=b2u[:, 0:1], in1=S[:, q * 512:(q + 1) * 512],
                        op0=OP.add, op1=OP.add)

            # --- software-pipelined flat loop -----------------------------
            # Stages are emitted deepest-offset first within each unit so
            # that every read of a rotating tile precedes the alloc of the
            # generation that reuses its buffer (WAR legality).
            # In-unit order: stall-prone update mms go LAST (in-order PE:
            # a waiting sn mm would block every later mm in the unit), and
            # h1g gelus land early in the Act queue. WAR legality needs
            # ms < h1 < ga.
            assert G >= 28, "modulo-schedule offsets need G >= 28"
            offs = (24, 19, 14, 10, 6, 0)
            stages = (up_stage, sc_stage, ms_stage, h1_stage, ga_stage,
                      st_stage)
            total = n_steps * G + offs[0] + 4
            for U in range(total):
                for off, fn in zip(offs, stages):
                    v = U - off
                    if v < 0:
                        continue
                    t, g = divmod(v, G)
                    if t < n_steps:
                        fn(t, g)

            # --- classifier head (transpose-free A-form) ------------------
            # Ac = S_g^T @ [cw1_s | cw1_t]  ->  [slots, 256];  then per-group
            # one-hot row-selects build hq[hid, 4] = cw1_s^T S[qs] +
            # cw1_t^T S[qt]; cb1 folds into the gelu bias (per-partition).
            nbatch = (G + 15) // 16
            for bq in range(nbatch):
                jn = min(16, G - bq * 16)
                pq = pMS.tile([P, 64], f32, tag="ms", name="pq")
                for j in range(jn):
                    g = bq * 16 + j
                    acp, act = ((pH1, "h1"), (pH2, "h1b"),
                                (pGA, "ga"))[j % 3]
                    ac = acp.tile([P, 256], f32, tag=act, name="ac")
                    mm(ac[:], lhsT=S[:, g * P:(g + 1) * P], rhs=cwst[:],
                       start=True, stop=True)
                    acs = wp.tile([P, 256], f16, tag="acs", name="acs")
                    if j % 2 == 0:
                        nc.vector.tensor_copy(acs[:], ac[:])
                    else:
                        nc.scalar.copy(acs[:], ac[:])
                    mm(pq[:, j * 4:(j + 1) * 4], lhsT=acs[:, 0:128],
                       rhs=qoh[:, g * 8:g * 8 + 4], start=True, stop=False)
                    mm(pq[:, j * 4:(j + 1) * 4], lhsT=acs[:, 128:256],
                       rhs=qoh[:, g * 8 + 4:g * 8 + 8],
                       start=False, stop=True)
                hg = wp.tile([P, 64], f32, tag="hg", name="hg")
                nc.scalar.activation(hg[:, 0:jn * 4], pq[:, 0:jn * 4],
                                     AF.Gelu, bias=cb1[:, 0:1])
                ops_ = pAG.tile([20, 64], f32, tag="ag", name="ops")
                mm(ops_[:, 0:jn * 4], lhsT=cw2[:], rhs=hg[:, 0:jn * 4],
                   start=True, stop=True)
                nc.scalar.activation(outsb[:, bq * 64:bq * 64 + jn * 4],
                                     ops_[:, 0:jn * 4], AF.Identity,
                                     bias=cb2[:, 0:1])
            nc.sync.dma_start(d_out[:], outsb[:])

    nc.finalize()
    return nc


def _assign_groups(ne, n_groups):
    """LPT bin packing: samples -> groups of <=GRP samples, balancing edge
    counts. Returns (group_of_sample, slot_of_sample, max_load)."""
    import heapq
    B = ne.shape[0]
    order = np.argsort(-ne, kind="stable")
    loads = [0] * n_groups
    counts = [0] * n_groups
    gof = np.zeros(B, np.int64)
    sof = np.zeros(B, np.int64)
    hp = [(0, g) for g in range(n_groups)]
    heapq.heapify(hp)
    for s in order:
        while True:
            ld, g = heapq.heappop(hp)
            if counts[g] < GRP and ld == loads[g]:
                break
        gof[s] = g
        sof[s] = counts[g]
        counts[g] += 1
        loads[g] += int(ne[s])
        if counts[g] < GRP:
            heapq.heappush(hp, (loads[g], g))
    return gof, sof, max(loads)


def _host_prep(inputs, G=None):
    f, hh = np.float32, np.float16
    src = np.asarray(inputs["edge_src"], np.int64)
    rel = np.asarray(inputs["edge_rel"], np.int64)
    tgt = np.asarray(inputs["edge_tgt"], np.int64)
    ne = np.asarray(inputs["n_edges"], np.int64)
    qs = np.asarray(inputs["query_src"], np.int64)
    qt = np.asarray(inputs["query_tgt"], np.int64)
    B = src.shape[0]
    if G is None:
        bc = -(-B // N_CORES)
        G = max(28, (-(-bc // GRP) + 3) // 4 * 4)
    while True:
        gof, sof, maxload = _assign_groups(ne, G * N_CORES)
        if maxload <= EC:
            break
        G += 4
    NG = G * N_CORES
    NQ = G // 4
    NP = G // 2

    oh = np.zeros((NG, P, 3 * EC), hh)
    relt4 = np.zeros((NG // 4, P, 512), hh)
    cnt = np.zeros(NG * P, np.float32)
    qoh = np.zeros((NG, P, 8), hh)
    ecnt = np.zeros(NG, np.int64)
    for s in range(B):
        g = int(gof[s])
        so = int(sof[s]) * N_ENT
        k = int(ne[s])
        qg, m4 = divmod(g, 4)
        rb = ((qg % (G // 4)) % 2) * 32
        if k:
            e0 = int(ecnt[g])
            ecnt[g] += k
            idx = np.arange(e0, e0 + k)
            es, et, er = src[s, :k], tgt[s, :k], rel[s, :k]
            oh[g, so + es, idx] = 1
            oh[g, so + et, EC + idx] = 1
            oh[g, idx, 2 * EC + so + et] = 1
            relt4[qg, rb + er, m4 * EC + idx] = 1
            np.add.at(cnt, g * P + so + et, 1.0)
        qoh[g, so + qs[s], sof[s]] = 1
        qoh[g, so + qt[s], 4 + sof[s]] = 1
    cntb = (np.asarray(inputs["msg_b2"], f).reshape(P, 1)
            * cnt[None, :]).astype(hh)

    # shared weights
    ee = np.asarray(inputs["entity_embed"], f)
    W1 = np.asarray(inputs["msg_W1"], f)
    reltab = (np.asarray(inputs["rel_embed"], f) @ W1[128:256]
              + np.asarray(inputs["msg_b1"], f))
    rt4 = np.zeros((P, 256), f)
    b2row = np.zeros((P, P), f)
    for rb in (0, 32, 64, 96):
        rt4[rb:rb + 20] = reltab
        b2row[rb] = np.asarray(inputs["msg_b2"], f)
    w2m_ = np.asarray(inputs["msg_W2"], f)
    w1u_ = np.asarray(inputs["upd_W1"], f)
    w2u_ = np.asarray(inputs["upd_W2"], f)
    cw1_ = np.asarray(inputs["cls_W1"], f)

    wf16 = np.concatenate([
        W1[0:128], W1[256:384], rt4,
        np.concatenate([w2m_[0:128], w2m_[128:256]], axis=1),
        np.concatenate(
            [w1u_[0:128, 0:128], w1u_[0:128, 128:256],
             w1u_[128:256, 0:128], w1u_[128:256, 128:256]], axis=1),
        np.concatenate([w2u_[0:128], w2u_[128:256]], axis=1),
        b2row,
        np.concatenate([cw1_[0:128], cw1_[128:256]], axis=1),
    ], axis=1).astype(hh)
    wf32 = np.concatenate([
        np.asarray(inputs["upd_b1"], f).reshape(2, 128).T,
        np.asarray(inputs["upd_b2"], f).reshape(128, 1),
        np.concatenate([cw1_[0:128], cw1_[128:256]], axis=1),
        np.asarray(inputs["cls_b1"], f).reshape(128, 1),
        np.asarray(inputs["cls_W2"], f),
        np.asarray(inputs["msg_b2"], f).reshape(128, 1),
    ], axis=1).astype(f)
    shared = {
        "s0": np.tile(ee.T, (1, GRP * G)).astype(hh),
        "sts0": np.tile(ee, (GRP, 1)).astype(hh),
        "wf16": wf16,
        "wf32": wf32,
        "cb2": np.asarray(inputs["cls_b2"], f).reshape(20, 1).copy(),
    }

    in_maps = []
    for c in range(N_CORES):
        gsl = slice(c * G, (c + 1) * G)
        psl = slice(c * NP, (c + 1) * NP)
        m = dict(shared)
        m["oh"] = np.ascontiguousarray(
            oh[gsl].reshape(NQ, 4, P, 3 * EC).transpose(0, 2, 1, 3)
            .reshape(NQ, P, 12 * EC))
        m["relt4"] = np.ascontiguousarray(
            relt4[c * NQ:(c + 1) * NQ].transpose(1, 0, 2)
            .reshape(P, NQ * 512))
        m["cntb"] = np.ascontiguousarray(
            cntb[:, c * G * P:(c + 1) * G * P])
        m["qoh"] = np.ascontiguousarray(
            qoh[gsl].transpose(1, 0, 2).reshape(P, G * 8))
        in_maps.append(m)
    return in_maps, gof, sof, G


_CACHE = {}


def kernel(**inputs):
    B = np.asarray(inputs["edge_src"]).shape[0]
    in_maps, gof, sof, G = _host_prep(inputs)

    key = G
    if key not in _CACHE:
        _CACHE[key] = _build_nc(G, N_STEPS)
    nc = _CACHE[key]

    from concourse.bass_utils import run_bass_kernel_spmd
    res = run_bass_kernel_spmd(nc, in_maps, core_ids=list(range(N_CORES)))

    out = np.empty((B, N_REL), np.float32)
    allc = np.concatenate([r["out"].T for r in res.results], axis=0)
    # row index in allc: core * (G*4) + (g_local*4 + slot) = gof*4 + sof
    out[:, :] = allc[gof * 4 + sof]
    return np.ascontiguousarray(out)



# revision 54
# speedup vs baseline: 1.0114x; 1.0008x over previous
"""Trainium2 Bass kernel: CLUTRR-style GNN message passing (nn_CLUTRRV4).

Data-parallel across 8 NeuronCores. Samples are packed 4-per-group
(4 x 32 entity slots = 128 partitions); sample->group assignment is an
LPT bin-packing so that each group's VALID edges fit in EC=128 packed
edge columns (vs 256 naive), skipping all masked-edge compute.

All one-hot gather/scatter/rel matrices are precomputed on the host and
DMA'd once (they are step-invariant). Entity state S is fp16.

Per step: S is transposed to slot-major (sync-queue DMA XBAR transpose
alternating with PE is_transpose matmuls; step 0 shares one host-sent
tile since S starts identical for every group), src/tgt states are
gathered via one-hot matmuls into a quad-wide gtb, the message MLP
layer 1 runs quad-wide (6 mms of N=512 per 4 groups, rel contribution
via a 20-row band), messages are scattered back edge-major, the msg_b2
bias folds into the PSUM->SBUF copy as a DVE scalar_tensor_tensor with
a host-precomputed b2*edge-count table, and the update MLP runs per
quad with N=512 matmuls. The classifier head is transpose-free
(A-form: Ac = S_g^T [cw1_s|cw1_t], then one-hot row-selects; cb1 rides
the gelu bias). Emission is a software-pipelined flat loop (modulo
schedule); PSUM is budgeted at exactly 8 banks.
"""
import sys
import numpy as np

if "/opt/trn_rl_repo" not in sys.path:
    sys.path.append("/opt/trn_rl_repo")

N_ENT, N_REL, D, E = 32, 20, 128, 64
N_STEPS = 8
N_CORES = 8
P = 128
EC = 128          # packed edge columns per group
GRP = 4           # samples per group



def _build_nc(G, n_steps):
    from concourse import bacc, mybir
    from concourse.tile import TileContext
    from concourse.masks import make_identity

    f32 = mybir.dt.float32
    f16 = mybir.dt.float16
    AF = mybir.ActivationFunctionType
    OP = mybir.AluOpType

    assert G % 4 == 0
    NQ = G // 4
    SLOTS = G * P

    nc = bacc.Bacc()

    def din(name, shape, dtype=f32):
        return nc.declare_dram_parameter(name, list(shape), dtype, isOutput=False)

    NP = G // 2
    d_s0 = din("s0", (P, SLOTS), f16)
    d_oh = din("oh", (NQ, P, 12 * EC), f16)     # 4 groups x [ohs|oht|ohe]
    d_relt4 = din("relt4", (P, NQ * 512), f16)  # quad-wide rel one-hots
    d_cntb = din("cntb", (P, G * P), f16)       # per-slot edge counts, bcast
    d_qoh = din("qoh", (P, G * 8), f16)
    d_sts0 = din("sts0", (P, P), f16)           # slot-major initial state
    d_wf16 = din("wf16", (P, 2176), f16)
    d_wf32 = din("wf32", (P, 281))
    d_cb2 = din("cb2", (20, 1))
    d_out = nc.declare_dram_parameter("out", [20, G * GRP], f32, isOutput=True)

    with TileContext(nc) as tc:
        with (
            tc.tile_pool(name="c", bufs=1) as cp,
            tc.tile_pool(name="w", bufs=4) as wp,
            tc.tile_pool(name="pGA", bufs=1, space="PSUM") as pGA,
            tc.tile_pool(name="pH1", bufs=1, space="PSUM") as pH1,
            tc.tile_pool(name="pMS", bufs=1, space="PSUM") as pMS,
            tc.tile_pool(name="pAG", bufs=1, space="PSUM") as pAG,
            tc.tile_pool(name="pUP", bufs=2, space="PSUM") as pUP,
            tc.tile_pool(name="pTR", bufs=1, space="PSUM") as pTR,
            tc.tile_pool(name="pH2", bufs=1, space="PSUM") as pH2,
        ):
            wf16 = cp.tile([P, 2176], f16, tag="wf16", name="wf16")
            w1s = wf16[:, 0:256]
            w1t = wf16[:, 256:512]
            rt4 = wf16[:, 512:768]
            w2m = wf16[:, 768:1024]
            w1u = wf16[:, 1024:1536]
            w2u = wf16[:, 1536:1792]
            b2row = wf16[:, 1792:1920]
            cwst = wf16[:, 1920:2176]
            wf32 = cp.tile([P, 281], f32, tag="wf32", name="wf32")
            nc.sync.dma_start(wf32[:], d_wf32[:])
            b1u = wf32[:, 0:2]
            b2u = wf32[:, 2:3]
            cw1 = wf32[:, 3:259]
            cb1 = wf32[:, 259:260]
            cw2 = wf32[:, 260:280]
            b2mc = wf32[:, 280:281]
            cb2 = cp.tile([20, 1], f32, tag="cb2", name="cb2")
            nc.sync.dma_start(cb2[:], d_cb2[:])

            ident = cp.tile([P, P], f16, tag="ident", name="ident")
            make_identity(nc, ident[:])

            # interleave S-quad and one-hot-quad DMAs so step-0 compute
            # never starves; rel/ind early (needed at h1/sc offsets)
            S = cp.tile([P, SLOTS], f16, tag="S", name="S")
            OH = cp.tile([P, G * 3 * EC], f16, tag="OH", name="OH")
            RELT4 = cp.tile([P, NQ * 512], f16, tag="RELT4", name="RELT4")
            CNTB = cp.tile([P, G * P], f16, tag="CNTB", name="CNTB")
            # step-0-critical tiles first: sts0 (shared t=0 slot-major
            # state) + OH/RELT2 chunks; S (first read by up, ~10 units in)
            # and CNTB (first read by sc) trail each quad's critical pair
            sts0 = cp.tile([P, P], f16, tag="sts0", name="sts0")
            nc.sync.dma_start(sts0[:], d_sts0[:])
            nc.sync.dma_start(OH[:, 0:1536], d_oh[0])
            nc.sync.dma_start(wf16[:], d_wf16[:])
            for q in range(NQ):
                if q > 0:
                    nc.sync.dma_start(OH[:, q * 1536:(q + 1) * 1536],
                                      d_oh[q])
                nc.sync.dma_start(RELT4[:, q * 512:(q + 1) * 512],
                                  d_relt4[:, q * 512:(q + 1) * 512])
                nc.sync.dma_start(S[:, q * 512:(q + 1) * 512],
                                  d_s0[:, q * 512:(q + 1) * 512])
                nc.sync.dma_start(CNTB[:, q * 512:(q + 1) * 512],
                                  d_cntb[:, q * 512:(q + 1) * 512])
            qoh = cp.tile([P, G * 8], f16, tag="qoh", name="qoh")
            nc.sync.dma_start(qoh[:], d_qoh[:])

            outsb = cp.tile([20, G * GRP], f32, tag="outsb", name="outsb")

            def ohs(g):
                return OH[:, g * 384:g * 384 + EC]

            def oht(g):
                return OH[:, g * 384 + EC:g * 384 + 2 * EC]

            def ohe(g):
                return OH[:, g * 384 + 2 * EC:g * 384 + 3 * EC]

            mm = nc.tensor.matmul
            st_ = {}
            sts_t, gtb_t, h1g_t, msb_t, agb_t = {}, {}, {}, {}, {}

            # --- pipeline stages ------------------------------------------
            def st_stage(t, g):
                # slot-major S replica via sync-queue DMA XBAR transpose.
                # At t=0 every group's S is the same tiled entity table ->
                # share one host-sent sts0 tile.
                if t == 0:
                    sts_t[g] = sts0
                    return
                t_ = wp.tile([P, P], f16, tag="sts", bufs=8, name="sts")
                if g % 2 == 0:
                    nc.sync.dma_start_transpose(t_[:], S[:, g * P:(g + 1) * P])
                else:
                    tp = pTR.tile([P, P], f16, tag="tr", name="tp")
                    mm(tp[:], lhsT=S[:, g * P:(g + 1) * P], rhs=ident[:],
                       is_transpose=True, start=True, stop=True)
                    nc.scalar.copy(t_[:], tp[:])
                sts_t[g] = t_

            def ga_stage(t, g):
                gi = g % 2
                if g % 4 == 0:
                    gtb_t[g // 4] = wp.tile([P, 1024], f16, tag="gtb",
                                            name="gtb")
                if gi == 0:
                    st_['gap'] = pGA.tile([P, 512], f32, tag="ga", name="gap")
                gp = st_['gap']
                mm(gp[:, gi * 256:(gi + 1) * 256],
                   lhsT=sts_t[g][:], rhs=OH[:, g * 384:g * 384 + 2 * EC],
                   start=True, stop=True)
                if gi == 1:
                    p = g // 2
                    gtb = gtb_t[g // 4]
                    dst = gtb[:, (p % 2) * 512:(p % 2) * 512 + 512]
                    # alternate engines: keeps the copy out of the DVE FIFO
                    # behind the heavy stt ops, so pGA (bufs=1) frees sooner
                    if p % 2 == 0:
                        nc.scalar.copy(dst, gp[:])
                    else:
                        nc.vector.tensor_copy(dst, gp[:])

            def h1_stage(t, g):
                # quad-wide: 6 mms of N=512 per 4 groups. gtb layout is
                # [g0s g0t g1s g1t g2s g2t g3s g3t]; strided AP views pick
                # the four src (resp tgt) blocks as the moving operand.
                if g % 4 != 3:
                    return
                q = g // 4
                rb = (q % 2) * 32
                gtb8 = gtb_t[q][:].rearrange("p (g s e) -> p g s e",
                                             g=4, s=2, e=EC)
                for c in (0, 1):
                    pool, tg = ((pH1, "h1") if c == 0 else (pH2, "h1b"))
                    hp = pool.tile([P, 512], f32, tag=tg, name="h1p")
                    mm(hp[:], lhsT=rt4[rb:rb + 20, c * P:(c + 1) * P],
                       rhs=RELT4[rb:rb + 20, q * 512:(q + 1) * 512],
                       start=True, stop=False, tile_position=(rb, 0))
                    mm(hp[:], lhsT=w1s[:, c * P:(c + 1) * P],
                       rhs=gtb8[:, :, 0, :], start=False, stop=False)
                    mm(hp[:], lhsT=w1t[:, c * P:(c + 1) * P],
                       rhs=gtb8[:, :, 1, :], start=False, stop=True)
                    t_ = wp.tile([P, 512], f16, tag="h1g", name="h1g")
                    nc.scalar.activation(t_[:], hp[:], AF.Gelu)
                    h1g_t[q * 2 + c] = t_

            def ms_stage(t, g):
                q, j = divmod(g, 4)
                if j == 0:
                    st_['msp'] = pMS.tile([P, 512], f32, tag="ms", name="msp")
                mp = st_['msp']
                o = mp[:, j * P:(j + 1) * P]
                for c in (0, 1):
                    h1g = h1g_t[q * 2 + c]
                    mm(o, lhsT=h1g[:, j * P:(j + 1) * P],
                       rhs=w2m[:, c * P:(c + 1) * P],
                       start=(c == 0), stop=(c == 1))
                if j == 3:
                    t_ = wp.tile([P, 512], f16, tag="msb", name="msb")
                    nc.vector.tensor_copy(t_[:], mp[:])
                    msb_t[g // 4] = t_

            def sc_stage(t, g):
                q, j = divmod(g, 4)
                if j == 0:
                    st_['agp'] = pAG.tile([P, 512], f32, tag="ag", name="agp")
                ap_ = st_['agp']
                msb = msb_t[q]
                o = ap_[:, j * P:(j + 1) * P]
                mm(o, lhsT=msb[:, j * P:(j + 1) * P], rhs=ohe(g),
                   start=True, stop=True)
                if j == 3:
                    # agb = agg + msg_b2 * per-slot valid-edge count, folded
                    # into the PSUM->SBUF copy (no PE b2 matmuls)
                    t_ = wp.tile([P, 512], f16, tag="agb", name="agb")
                    nc.vector.scalar_tensor_tensor(
                        out=t_[:], in0=ap_[:], scalar=1.0,
                        in1=CNTB[:, q * 512:(q + 1) * 512],
                        op0=OP.mult, op1=OP.add)
                    agb_t[q] = t_

            def up_stage(t, w):
                q, ph = divmod(w, 4)
                if ph == 0:
                    h3a = pUP.tile([P, 512], f32, tag="up", name="h3a")
                    h3b = pUP.tile([P, 512], f32, tag="up", name="h3b")
                    st_['h3a'], st_['h3b'] = h3a, h3b
                    for mc, hb in ((0, h3a), (1, h3b)):
                        mm(hb[:], lhsT=w1u[:, mc * P:(mc + 1) * P],
                           rhs=S[:, q * 512:(q + 1) * 512],
                           start=True, stop=False)
                        mm(hb[:], lhsT=w1u[:, 256 + mc * P:256 + (mc + 1) * P],
                           rhs=agb_t[q][:], start=False, stop=True)
                elif ph == 1:
                    t_ = wp.tile([P, 1024], f16, tag="h3g", name="h3g")
                    nc.scalar.activation(t_[:, 0:512], st_['h3a'][:], AF.Gelu,
                                         bias=b1u[:, 0:1])
                    st_['h3g'] = t_
                elif ph == 2:
                    nc.scalar.activation(st_['h3g'][:, 512:1024],
                                         st_['h3b'][:], AF.Gelu,
                                         bias=b1u[:, 1:2])
                else:
                    sn = pUP.tile([P, 512], f32, tag="up", name="sn")
                    for kc in (0, 1):
                        mm(sn[:], lhsT=w2u[:, kc * P:(kc + 1) * P],
                           rhs=st_['h3g'][:, kc * 512:(kc + 1) * 512],
                           start=(kc == 0), stop=(kc == 1))
                    nc.vector.scalar_tensor_tensor(
                        out=S[:, q * 512:(q + 1) * 512], in0=sn[:],
                        scalar=b2u[:, 0:1], in1=S[:, q * 512:(q + 1) * 512],
                        op0=OP.add, op1=OP.add)

            # --- software-pipelined flat loop -----------------------------
            # Stages are emitted deepest-offset first within each unit so
            # that every read of a rotating tile precedes the alloc of the
            # generation that reuses its buffer (WAR legality).
            # In-unit order: stall-prone update mms go LAST (in-order PE:
            # a waiting sn mm would block every later mm in the unit), and
            # h1g gelus land early in the Act queue. WAR legality needs
            # ms < h1 < ga.
            assert G >= 28, "modulo-schedule offsets need G >= 28"
            offs = (24, 19, 14, 10, 6, 0)
            stages = (up_stage, sc_stage, ms_stage, h1_stage, ga_stage,
                      st_stage)
            total = n_steps * G + offs[0] + 4
            for U in range(total):
                for off, fn in zip(offs, stages):
                    v = U - off
                    if v < 0:
                        continue
                    t, g = divmod(v, G)
                    if t < n_steps:
                        fn(t, g)

            # --- classifier head (transpose-free A-form) ------------------
            # Ac = S_g^T @ [cw1_s | cw1_t]  ->  [slots, 256];  then per-group
            # one-hot row-selects build hq[hid, 4] = cw1_s^T S[qs] +
            # cw1_t^T S[qt]; cb1 folds into the gelu bias (per-partition).
            nbatch = (G + 15) // 16
            for bq in range(nbatch):
                jn = min(16, G - bq * 16)
                pq = pMS.tile([P, 64], f32, tag="ms", name="pq")
                for j in range(jn):
                    g = bq * 16 + j
                    acp, act = ((pH1, "h1"), (pH2, "h1b"),
                                (pGA, "ga"))[j % 3]
                    ac = acp.tile([P, 256], f32, tag=act, name="ac")
                    mm(ac[:], lhsT=S[:, g * P:(g + 1) * P], rhs=cwst[:],
                       start=True, stop=True)
                    acs = wp.tile([P, 256], f16, tag="acs", name="acs")
                    if j % 2 == 0:
                        nc.vector.tensor_copy(acs[:], ac[:])
                    else:
                        nc.scalar.copy(acs[:], ac[:])
                    mm(pq[:, j * 4:(j + 1) * 4], lhsT=acs[:, 0:128],
                       rhs=qoh[:, g * 8:g * 8 + 4], start=True, stop=False)
                    mm(pq[:, j * 4:(j + 1) * 4], lhsT=acs[:, 128:256],
                       rhs=qoh[:, g * 8 + 4:g * 8 + 8],
                       start=False, stop=True)
                hg = wp.tile([P, 64], f32, tag="hg", name="hg")
                nc.scalar.activation(hg[:, 0:jn * 4], pq[:, 0:jn * 4],
                                     AF.Gelu, bias=cb1[:, 0:1])
                ops_ = pAG.tile([20, 64], f32, tag="ag", name="ops")
                mm(ops_[:, 0:jn * 4], lhsT=cw2[:], rhs=hg[:, 0:jn * 4],
                   start=True, stop=True)
                nc.scalar.activation(outsb[:, bq * 64:bq * 64 + jn * 4],
                                     ops_[:, 0:jn * 4], AF.Identity,
                                     bias=cb2[:, 0:1])
                nc.sync.dma_start(d_out[:, bq * 64:bq * 64 + jn * 4],
                                  outsb[:, bq * 64:bq * 64 + jn * 4])

    nc.finalize()
    return nc


def _assign_groups(ne, n_groups):
    """LPT bin packing: samples -> groups of <=GRP samples, balancing edge
    counts. Returns (group_of_sample, slot_of_sample, max_load)."""
    import heapq
    B = ne.shape[0]
    order = np.argsort(-ne, kind="stable")
    loads = [0] * n_groups
    counts = [0] * n_groups
    gof = np.zeros(B, np.int64)
    sof = np.zeros(B, np.int64)
    hp = [(0, g) for g in range(n_groups)]
    heapq.heapify(hp)
    for s in order:
        while True:
            ld, g = heapq.heappop(hp)
            if counts[g] < GRP and ld == loads[g]:
                break
        gof[s] = g
        sof[s] = counts[g]
        counts[g] += 1
        loads[g] += int(ne[s])
        if counts[g] < GRP:
            heapq.heappush(hp, (loads[g], g))
    return gof, sof, max(loads)


def _host_prep(inputs, G=None):
    f, hh = np.float32, np.float16
    src = np.asarray(inputs["edge_src"], np.int64)
    rel = np.asarray(inputs["edge_rel"], np.int64)
    tgt = np.asarray(inputs["edge_tgt"], np.int64)
    ne = np.asarray(inputs["n_edges"], np.int64)
    qs = np.asarray(inputs["query_src"], np.int64)
    qt = np.asarray(inputs["query_tgt"], np.int64)
    B = src.shape[0]
    if G is None:
        bc = -(-B // N_CORES)
        G = max(28, (-(-bc // GRP) + 3) // 4 * 4)
    while True:
        gof, sof, maxload = _assign_groups(ne, G * N_CORES)
        if maxload <= EC:
            break
        G += 4
    NG = G * N_CORES
    NQ = G // 4
    NP = G // 2

    oh = np.zeros((NG, P, 3 * EC), hh)
    relt4 = np.zeros((NG // 4, P, 512), hh)
    cnt = np.zeros(NG * P, np.float32)
    qoh = np.zeros((NG, P, 8), hh)
    ecnt = np.zeros(NG, np.int64)
    for s in range(B):
        g = int(gof[s])
        so = int(sof[s]) * N_ENT
        k = int(ne[s])
        qg, m4 = divmod(g, 4)
        rb = ((qg % (G // 4)) % 2) * 32
        if k:
            e0 = int(ecnt[g])
            ecnt[g] += k
            idx = np.arange(e0, e0 + k)
            es, et, er = src[s, :k], tgt[s, :k], rel[s, :k]
            oh[g, so + es, idx] = 1
            oh[g, so + et, EC + idx] = 1
            oh[g, idx, 2 * EC + so + et] = 1
            relt4[qg, rb + er, m4 * EC + idx] = 1
            np.add.at(cnt, g * P + so + et, 1.0)
        qoh[g, so + qs[s], sof[s]] = 1
        qoh[g, so + qt[s], 4 + sof[s]] = 1
    cntb = (np.asarray(inputs["msg_b2"], f).reshape(P, 1)
            * cnt[None, :]).astype(hh)

    # shared weights
    ee = np.asarray(inputs["entity_embed"], f)
    W1 = np.asarray(inputs["msg_W1"], f)
    reltab = (np.asarray(inputs["rel_embed"], f) @ W1[128:256]
              + np.asarray(inputs["msg_b1"], f))
    rt4 = np.zeros((P, 256), f)
    b2row = np.zeros((P, P), f)
    for rb in (0, 32, 64, 96):
        rt4[rb:rb + 20] = reltab
        b2row[rb] = np.asarray(inputs["msg_b2"], f)
    w2m_ = np.asarray(inputs["msg_W2"], f)
    w1u_ = np.asarray(inputs["upd_W1"], f)
    w2u_ = np.asarray(inputs["upd_W2"], f)
    cw1_ = np.asarray(inputs["cls_W1"], f)

    wf16 = np.concatenate([
        W1[0:128], W1[256:384], rt4,
        np.concatenate([w2m_[0:128], w2m_[128:256]], axis=1),
        np.concatenate(
            [w1u_[0:128, 0:128], w1u_[0:128, 128:256],
             w1u_[128:256, 0:128], w1u_[128:256, 128:256]], axis=1),
        np.concatenate([w2u_[0:128], w2u_[128:256]], axis=1),
        b2row,
        np.concatenate([cw1_[0:128], cw1_[128:256]], axis=1),
    ], axis=1).astype(hh)
    wf32 = np.concatenate([
        np.asarray(inputs["upd_b1"], f).reshape(2, 128).T,
        np.asarray(inputs["upd_b2"], f).reshape(128, 1),
        np.concatenate([cw1_[0:128], cw1_[128:256]], axis=1),
        np.asarray(inputs["cls_b1"], f).reshape(128, 1),
        np.asarray(inputs["cls_W2"], f),
        np.asarray(inputs["msg_b2"], f).reshape(128, 1),
    ], axis=1).astype(f)
    shared = {
        "s0": np.tile(ee.T, (1, GRP * G)).astype(hh),
        "sts0": np.tile(ee, (GRP, 1)).astype(hh),
        "wf16": wf16,
        "wf32": wf32,
        "cb2": np.asarray(inputs["cls_b2"], f).reshape(20, 1).copy(),
    }

    in_maps = []
    for c in range(N_CORES):
        gsl = slice(c * G, (c + 1) * G)
        psl = slice(c * NP, (c + 1) * NP)
        m = dict(shared)
        m["oh"] = np.ascontiguousarray(
            oh[gsl].reshape(NQ, 4, P, 3 * EC).transpose(0, 2, 1, 3)
            .reshape(NQ, P, 12 * EC))
        m["relt4"] = np.ascontiguousarray(
            relt4[c * NQ:(c + 1) * NQ].transpose(1, 0, 2)
            .reshape(P, NQ * 512))
        m["cntb"] = np.ascontiguousarray(
            cntb[:, c * G * P:(c + 1) * G * P])
        m["qoh"] = np.ascontiguousarray(
            qoh[gsl].transpose(1, 0, 2).reshape(P, G * 8))
        in_maps.append(m)
    return in_maps, gof, sof, G


_CACHE = {}


def kernel(**inputs):
    B = np.asarray(inputs["edge_src"]).shape[0]
    in_maps, gof, sof, G = _host_prep(inputs)

    key = G
    if key not in _CACHE:
        _CACHE[key] = _build_nc(G, N_STEPS)
    nc = _CACHE[key]

    from concourse.bass_utils import run_bass_kernel_spmd
    res = run_bass_kernel_spmd(nc, in_maps, core_ids=list(range(N_CORES)))

    out = np.empty((B, N_REL), np.float32)
    allc = np.concatenate([r["out"].T for r in res.results], axis=0)
    # row index in allc: core * (G*4) + (g_local*4 + slot) = gof*4 + sof
    out[:, :] = allc[gof * 4 + sof]
    return np.ascontiguousarray(out)

